# revision 6
# baseline (speedup 1.0000x reference)
"""AdaptiveSpectrumLayer Trainium2 kernel v3 (8-core data-parallel).

v2 -> v3 (critical-path fixes from TimelineSim gap analysis):
  * features via Abs_reciprocal_sqrt (direct 1/sqrt on Act): rsq feeds
    both mag = sq*rsq (DVE) and sin/cos = T*rsq (DVE) in parallel;
    ssq and sincos are split per re/im pair so each starts as soon as
    its DFT copies land.
  * gate K-order [nyq3, mag_a, mag_b, sin_a, sin_b, cos_a, cos_b]:
    early-ready features first, so PE doesn't stall mid-accumulation.
  * deferred softmax normalization (as v1): inverse DFT runs on
    UNNORMALIZED U = T*e right after exp; the expsum->recip->broadcast
    chain overlaps it and only the final zo = psz*rb multiply needs rb.
  * all small M=1 matmul outputs (nyq re256 x2, gate row-256 x2,
    expsum x2) packed into disjoint partition rows of one shared PSUM
    tile: frees PSUM banks for cross-rep overlap.
  * act-table prefetch: dummy ops keyed to late-phase products trigger
    the sqrt<->exp table loads off the critical path.
"""

import numpy as np

B, H, F = 128, 512, 64
HID = 16
NF = H // 2 + 1          # 257
NCORES = 8
BL = B // NCORES         # 16 batch per core
BF = BL * F              # 1024 free columns per core
P = 128
CH = 512                 # free-dim chunk (8 batches x 64)
NCH = BF // CH           # 2


def _bf16():
    import ml_dtypes
    return ml_dtypes.bfloat16


def _build_constants(W_proj, b_proj, W_gate, b_gate):
    W_proj = np.asarray(W_proj, np.float64)
    b_proj = np.asarray(b_proj, np.float64)
    W_gate = np.asarray(W_gate, np.float64)
    b_gate = np.asarray(b_gate, np.float64)

    Wg = W_gate.reshape(NF, NF, HID)                      # [m, n, h]
    A = np.einsum("nch,mnh->ncm", W_proj, Wg)             # (257, 3, 257)
    bias_eff = b_gate + np.einsum("nh,mnh->m", b_proj, Wg)

    h = np.arange(H)
    n = np.arange(NF)
    ang = 2.0 * np.pi * np.outer(h, n) / H                # (512, 257)
    Cf = np.cos(ang)
    Sf = -np.sin(ang)

    # radix-2: even freqs contract with xp = x[:256]+x[256:], odd with
    # xm = x[:256]-x[256:]; freq tiles are parity-permuted everywhere:
    pa = np.arange(0, 256, 2)    # "a" tile: even freqs 0..254
    pb = np.arange(1, 256, 2)    # "b" tile: odd freqs 1..255
    # forward DFT weights (K = h<256), M-order [im_a im_b re_a re_b re256]
    Wf = np.concatenate(
        [Sf[0:256][:, pa], Sf[0:256][:, pb],
         Cf[0:256][:, pa], Cf[0:256][:, pb], Cf[0:256, 256:257]],
        axis=1,
    )                                                     # (256, 513)
    Wf_p = np.ascontiguousarray(
        Wf.reshape(2, P, 513).transpose(1, 0, 2)
    ).astype(_bf16())                                     # (P, 2, 513)

    # gate weights: K-tiles [mag_a mag_b sin_a sin_b cos_a cos_b |r| sgn].
    # Output m-columns are parity-permuted too, so the logit Mtiles line
    # up with the parity-ordered fft tiles in the U = T*e multiply.
    mperm = np.concatenate([pa, pb, [256]])
    Ap = np.zeros((8, P, NF))
    Ap[0] = A[pa, 0, :][:, mperm]
    Ap[1] = A[pb, 0, :][:, mperm]
    Ap[2] = A[pa, 1, :][:, mperm]
    Ap[3] = A[pb, 1, :][:, mperm]
    Ap[4] = A[pa, 2, :][:, mperm]
    Ap[5] = A[pb, 2, :][:, mperm]
    Ap[2, 0] = bias_eff[mperm]   # sin(freq 0) == 0: row rides the bias ones
    Ap[6, 0] = A[256, 0, mperm]  # rhs row = |re256|   (K=1)
    Ap[7, 0] = A[256, 2, mperm]  # rhs row = sign(re256) (K=1)
    Ap_p = np.ascontiguousarray(Ap.transpose(1, 0, 2)).astype(_bf16())

    # inverse DFT weights: K-tiles [re_a re_b (nyq+im_a) im_b]
    cn = np.full(NF, 2.0)
    cn[0] = 1.0
    cn[256] = 1.0
    Ci = np.cos(ang) * cn[None, :] / H                    # (512, 257)
    Si = (-2.0 / H) * np.sin(ang)
    Wi = np.zeros((4, P, H))
    Wi[0] = Ci[:, pa].T
    Wi[1] = Ci[:, pb].T
    Wi[2, 0] = Ci[:, 256]                                 # Nyquist row
    Wi[2, 1:128] = Si[:, pa[1:]].T
    Wi[3] = Si[:, pb].T
    Wi_p = np.ascontiguousarray(Wi.transpose(1, 0, 2)).astype(_bf16())
    return Wf_p, Ap_p, Wi_p


def _build_graph(reps=1):
    from contextlib import ExitStack

    import concourse.bass as bass  # noqa
    import concourse.tile as tile
    from concourse import bacc, mybir

    F32 = mybir.dt.float32
    BF16 = mybir.dt.bfloat16
    AF = mybir.ActivationFunctionType

    nc = bacc.Bacc(
        "TRN2",
        target_bir_lowering=False,
        debug=False,
        num_devices=NCORES,
    )

    # partition-major DRAM layouts -> fully contiguous DMAs
    x_ext = nc.dram_tensor("x", [P, 4, BL, F], BF16, kind="ExternalInput").ap()
    wf_ext = nc.dram_tensor("wf", [P, 2, 513], BF16, kind="ExternalInput").ap()
    ap_ext = nc.dram_tensor("apk", [P, 8, NF], BF16, kind="ExternalInput").ap()
    wi_ext = nc.dram_tensor("wi", [P, 4, H], BF16, kind="ExternalInput").ap()
    out_ext = nc.dram_tensor("out", [P, NCH, 4, CH], BF16,
                             kind="ExternalOutput").ap()

    with tile.TileContext(nc) as tc, ExitStack() as ctx:
        const = ctx.enter_context(tc.tile_pool(name="const", bufs=1))
        xpool = ctx.enter_context(tc.tile_pool(name="xp", bufs=2))
        tpool = ctx.enter_context(tc.tile_pool(name="tp", bufs=2))
        fpool = ctx.enter_context(tc.tile_pool(name="fe", bufs=2))
        wpool = ctx.enter_context(tc.tile_pool(name="work", bufs=2))
        opool = ctx.enter_context(tc.tile_pool(name="outs", bufs=2))
        psmm = ctx.enter_context(tc.tile_pool(name="psmm", bufs=3, space="PSUM"))
        psy = ctx.enter_context(tc.tile_pool(name="psy", bufs=3, space="PSUM"))
        pssm = ctx.enter_context(tc.tile_pool(name="pssm", bufs=2, space="PSUM"))

        # ---- constants (single contiguous DMAs, cold path only)
        wf_sb = const.tile([P, 2, 513], BF16, tag="wf", name="wf")
        nc.sync.dma_start(wf_sb[:], wf_ext)
        ap_sb = const.tile([P, 8, NF], BF16, tag="apk", name="apk")
        nc.scalar.dma_start(ap_sb[:], ap_ext)
        wi_sb = const.tile([P, 4, H], BF16, tag="wi", name="wi")
        nc.sync.dma_start(wi_sb[:], wi_ext)

        ones_bf = const.tile([P, 1], BF16, tag="ones_bf", name="ones_bf")
        nc.vector.memset(ones_bf[:], 1.0)
        ones_row = const.tile([1, P], BF16, tag="ones_row", name="ones_row")
        nc.vector.memset(ones_row[:], 1.0)
        ones_ch = const.tile([1, CH], BF16, tag="ones_ch", name="ones_ch")
        nc.vector.memset(ones_ch[:], 1.0)
        warm = const.tile([1, 8], F32, tag="warm", name="warm")
        nc.scalar.activation(warm[:], ones_ch[0:1, 0:8],
                             func=AF.Abs_reciprocal_sqrt)

        for _rep in range(reps):
            x_sb = xpool.tile([P, 4, BL, F], BF16, tag="x", name="x")
            nc.sync.dma_start(x_sb[:], x_ext)
            # radix-2 pre-combine on the (otherwise idle) Pool engine
            xp = xpool.tile([P, 2, BL, F], BF16, tag="xp", name="xp", bufs=2)
            xm = xpool.tile([P, 2, BL, F], BF16, tag="xm", name="xm", bufs=2)
            nc.gpsimd.tensor_add(xp[:], x_sb[:, 0:2], x_sb[:, 2:4])
            nc.gpsimd.tensor_sub(xm[:], x_sb[:, 0:2], x_sb[:, 2:4])

            fts, feats_l, r256s = [], [], []

            # ============ phase A: forward DFT + squares =========
            for c in range(NCH):
                bsl = slice(c * (CH // F), (c + 1) * (CH // F))
                # ft = [im_a im_b re_a re_b]
                ft = tpool.tile([P, 4, CH], BF16, tag=f"ft{c}", name=f"ft{c}")
                ssq = wpool.tile([P, 4, CH], BF16, tag=f"ssq{c}", name=f"ssq{c}")
                for mt in range(4):
                    src_t = [xp, xm, xp, xm][mt]    # even<-xp, odd<-xm
                    ps = psmm.tile([P, CH], F32, tag="mm", name="mm")
                    for k in range(2):
                        nc.tensor.matmul(
                            ps[:],
                            wf_sb[:, k, mt * P:(mt + 1) * P],
                            src_t[:, k, bsl, :],
                            start=(k == 0),
                            stop=(k == 1),
                        )
                    eng = [nc.vector, nc.scalar, nc.scalar, nc.scalar][mt]
                    if eng is nc.scalar:
                        nc.scalar.activation(ft[:, mt, :], ps[:], func=AF.Copy)
                    else:
                        nc.vector.tensor_copy(ft[:, mt, :], ps[:])
                    if mt % 2 == 1:
                        # square the (a,b) pair as soon as both copies land
                        nc.vector.tensor_mul(
                            ssq[:, mt - 1:mt + 1, :],
                            ft[:, mt - 1:mt + 1, :], ft[:, mt - 1:mt + 1, :])

                # nyquist re256 row (M=1, even freq)
                psn = pssm.tile([1, CH], F32, tag="sm", name="psn")
                for k in range(2):
                    nc.tensor.matmul(
                        psn[:],
                        wf_sb[:, k, 512:513],
                        xp[:, k, bsl, :],
                        start=(k == 0),
                        stop=(k == 1),
                    )
                t_abs = wpool.tile([1, CH], BF16, tag=f"tabs{c}", name=f"tabs{c}")
                nc.scalar.activation(t_abs[:], psn[:], func=AF.Abs)
                t_sgn = wpool.tile([1, CH], BF16, tag=f"tsgn{c}", name=f"tsgn{c}")
                nc.scalar.activation(t_sgn[:], psn[:], func=AF.Sign)
                r256 = wpool.tile([1, CH], BF16, tag=f"r256_{c}", name=f"r256_{c}")
                nc.scalar.activation(r256[:], psn[:], func=AF.Copy)
                r256s.append((r256, t_abs, t_sgn))

                # features: rsq = 1/sqrt(sq); mag = sq*rsq; sin/cos = T*rsq
                sq = wpool.tile([P, 2, CH], BF16, tag=f"sq{c}", name=f"sq{c}")
                nc.vector.tensor_add(sq[:], ssq[:, 0:2, :], ssq[:, 2:4, :])
                rsq = wpool.tile([P, 2, CH], BF16, tag=f"rsq{c}", name=f"rsq{c}")
                nc.scalar.activation(rsq[:], sq[:], func=AF.Abs_reciprocal_sqrt)
                feats = fpool.tile([P, 6, CH], BF16, tag=f"fe{c}", name=f"fe{c}")
                nc.vector.tensor_mul(feats[:, 0:2, :], sq[:], rsq[:])
                nc.vector.tensor_mul(feats[:, 2:4, :], ft[:, 0:2, :], rsq[:])
                nc.vector.tensor_mul(feats[:, 4:6, :], ft[:, 2:4, :], rsq[:])
                # bias rides the always-zero sin(freq0) feature row
                nc.vector.tensor_copy(feats[0:1, 2, :], ones_ch[:])
                fts.append(ft)
                feats_l.append(feats)


            # == phase B/C per chunk: logits, weights, u, inverse ==
            zo = opool.tile([P, NCH, 4, CH], BF16, tag="zo", name="zo", bufs=1)
            for c in range(NCH):
                ft, feats = fts[c], feats_l[c]
                r256, t_abs, t_sgn = r256s[c]
                et = wpool.tile([P, 2, CH], BF16, tag=f"et{c}", name=f"et{c}")
                e2 = wpool.tile([1, CH], BF16, tag=f"e2_{c}", name=f"e2_{c}")
                # gate K-order: early-ready nyquist K=1 rows first
                klist = [(6, t_abs[0:1, :], 1), (7, t_sgn[0:1, :], 1)] + [
                    (k, feats[:, k, :], P) for k in range(6)]
                for mt in range(3):
                    msl = slice(mt * P, NF if mt == 2 else (mt + 1) * P)
                    mp = 1 if mt == 2 else P
                    ps = (pssm.tile([1, CH], F32, tag="sm", name="psy2")[:]
                          if mt == 2 else
                          psy.tile([mp, CH], F32, tag="y", name="psy")[:])
                    for i, (k, rhs, kk) in enumerate(klist):
                        nc.tensor.matmul(
                            ps, ap_sb[0:kk, k, msl], rhs,
                            start=(i == 0), stop=(i == len(klist) - 1))
                    # e = exp(silu(y)) via tanh: exp(0.5*(1+tanh(y/2))*y)
                    th = wpool.tile([mp, CH], F32, tag=f"th{mt}", name=f"th{mt}",
                                    bufs=3)
                    nc.scalar.activation(th[:], ps, func=AF.Tanh, scale=0.5)
                    ysw = wpool.tile([mp, CH], F32, tag=f"ysw{mt}",
                                     name=f"ysw{mt}", bufs=3)
                    nc.vector.scalar_tensor_tensor(
                        out=ysw[:], in0=th[:], scalar=1.0, in1=ps,
                        op0=mybir.AluOpType.add, op1=mybir.AluOpType.mult,
                    )
                    tgt = e2[:] if mt == 2 else et[:, mt, :]
                    nc.scalar.activation(tgt, ysw[:], func=AF.Exp, scale=0.5)

                # unnormalized U = T*e right after exp (inverse DFT needn't
                # wait for the softmax sum)
                Ure = wpool.tile([P, 2, CH], BF16, tag=f"ur{c}", name=f"ur{c}")
                nc.vector.tensor_mul(Ure[:], ft[:, 2:4, :], et[:])
                Uim = wpool.tile([P, 2, CH], BF16, tag=f"ui{c}", name=f"ui{c}")
                nc.vector.tensor_mul(Uim[:], ft[:, 0:2, :], et[:])
                nc.vector.tensor_mul(Uim[0:1, 0, :], r256[:], e2[:])
                U = [Ure[:, 0, :], Ure[:, 1, :], Uim[:, 0, :], Uim[:, 1, :]]

                # exp-sum, reciprocal, broadcast (overlaps inverse DFT)
                ps_s = pssm.tile([1, CH], F32, tag="sm", name="ps_s")
                nc.tensor.matmul(ps_s[:], ones_bf[:], et[:, 0, :],
                                 start=True, stop=False)
                nc.tensor.matmul(ps_s[:], ones_bf[:], et[:, 1, :],
                                 start=False, stop=False)
                nc.tensor.matmul(ps_s[:], ones_bf[0:1, :], e2[:],
                                 start=False, stop=True)
                srec = wpool.tile([1, CH], BF16, tag=f"srec_{c}",
                                  name=f"srec_{c}")
                with nc.allow_low_precision(reason="bf16 softmax scale"):
                    nc.vector.reciprocal(srec[:], ps_s[:])
                ps_rb = psmm.tile([P, CH], F32, tag="mm", name="ps_rb")
                nc.tensor.matmul(ps_rb[:], ones_row[:], srec[:], start=True,
                                 stop=True)
                rb = wpool.tile([P, CH], BF16, tag=f"rb_{c}", name=f"rb_{c}")
                nc.scalar.activation(rb[:], ps_rb[:], func=AF.Copy)

                # inverse DFT + deferred-normalization epilogue + out DMA
                ikorder = [0, 1, 3, 2]  # im_a (with late u256 row) last
                for mt in range(4):
                    ps = psmm.tile([P, CH], F32, tag="mm", name="psz")
                    for j, k in enumerate(ikorder):
                        nc.tensor.matmul(
                            ps[:],
                            wi_sb[:, k, mt * P:(mt + 1) * P],
                            U[k],
                            start=(j == 0),
                            stop=(j == 3),
                        )
                    nc.vector.tensor_mul(zo[:, c, mt, :], ps[:], rb[:])
                nc.sync.dma_start(out_ext[:, c], zo[:, c])

            # table prefetch: next rep's rsq-set load keyed to last exp
            nc.scalar.activation(warm[:], e2[0:1, 0:8],
                                 func=AF.Abs_reciprocal_sqrt)

    nc.compile()
    return nc


_CACHE = {}


def _pack_in_maps(inputs):
    Wf, Ap, Wi = _build_constants(
        inputs["W_proj"], inputs["b_proj"], inputs["W_gate"], inputs["b_gate"]
    )
    x = np.ascontiguousarray(np.asarray(inputs["x"], np.float32))
    return [
        {
            # (BL,H,F) -> (P,4,BL,F): partition-major, fully contiguous
            "x": np.ascontiguousarray(
                x[c * BL:(c + 1) * BL].transpose(1, 0, 2)
                .reshape(4, P, BL, F).transpose(1, 0, 2, 3)
            ).astype(_bf16()),
            "wf": Wf,
            "apk": Ap,
            "wi": Wi,
        }
        for c in range(NCORES)
    ]


def _unpack_out(r):
    # (P,NCH,4,CH) -> (BL,H,F): h = mt*128+p, b = c*8+bb, col = bb*64+f
    o = np.asarray(r, dtype=np.float32)
    return o.reshape(P, NCH, 4, CH // F, F).transpose(1, 3, 2, 0, 4).reshape(
        BL, H, F)


def _run(inputs, trace=False):
    from concourse.bass_utils import run_bass_kernel_spmd

    if "graph" not in _CACHE:
        _CACHE["graph"] = _build_graph()
    nc = _CACHE["graph"]
    in_maps = _pack_in_maps(inputs)
    res = run_bass_kernel_spmd(nc, in_maps, core_ids=list(range(NCORES)),
                               trace=trace)
    out = np.concatenate([_unpack_out(r["out"]) for r in res.results], axis=0)
    return out.astype(np.float32), res


def kernel(**inputs):
    out, _ = _run(inputs, trace=False)
    return out


def _make_exec(nc):
    """Build a jit-cached 8-core executor for a compiled Bacc graph,
    replicating bass2jax.run_bass_via_pjrt's multi-core path but reusable
    across calls (for timing)."""
    import jax
    import numpy as np
    from jax.sharding import Mesh, PartitionSpec
    from jax.experimental.shard_map import shard_map
    from concourse import mybir
    from concourse.bass2jax import _bass_exec_p, install_neuronx_cc_hook

    install_neuronx_cc_hook()
    from concourse.bass2jax import partition_id_tensor

    n_cores = NCORES
    pid_name = nc.partition_id_tensor.name if nc.partition_id_tensor else None
    in_names, out_names, out_avals, zero_outs = [], [], [], []
    for alloc in nc.m.functions[0].allocations:
        if not isinstance(alloc, mybir.MemoryLocationSet):
            continue
        name = alloc.memorylocations[0].name
        if alloc.kind == "ExternalInput":
            if name != pid_name:
                in_names.append(name)
        elif alloc.kind == "ExternalOutput":
            out_names.append(name)
            shape = tuple(alloc.tensor_shape)
            dtype = mybir.dt.np(alloc.dtype)
            out_avals.append(jax.core.ShapedArray(shape, dtype))
            zero_outs.append(np.zeros(shape, dtype))
    n_params = len(in_names)
    all_names = in_names + out_names
    if pid_name is not None:
        all_names = all_names + [pid_name]

    def _body(*args):
        operands = list(args)
        if pid_name is not None:
            operands.append(partition_id_tensor())
        outs = _bass_exec_p.bind(
            *operands,
            out_avals=tuple(out_avals),
            in_names=tuple(all_names),
            out_names=tuple(out_names),
            lowering_input_output_aliases=(),
            sim_require_finite=True,
            sim_require_nnan=True,
            nc=nc,
        )
        return tuple(outs)

    devices = jax.devices()[:n_cores]
    mesh = Mesh(np.asarray(devices), ("core",))
    n_all = n_params + len(out_names)
    fn = jax.jit(
        shard_map(
            _body,
            mesh=mesh,
            in_specs=(PartitionSpec("core"),) * n_all,
            out_specs=(PartitionSpec("core"),) * len(out_names),
            check_rep=False,
        ),
        keep_unused=True,
    )

    def pack(in_maps):
        concat = [
            np.concatenate([np.asarray(in_maps[c][k]) for c in range(n_cores)], axis=0)
            for k in in_names
        ]
        concat += [
            np.zeros((n_cores * z.shape[0], *z.shape[1:]), z.dtype) for z in zero_outs
        ]
        return [jax.device_put(a) for a in concat]

    return fn, pack, out_names, out_avals


# revision 7
# speedup vs baseline: 2.0007x; 2.0007x over previous
"""AdaptiveSpectrumLayer Trainium2 kernel v3 (8-core data-parallel).

v2 -> v3 (critical-path fixes from TimelineSim gap analysis):
  * features via Abs_reciprocal_sqrt (direct 1/sqrt on Act): rsq feeds
    both mag = sq*rsq (DVE) and sin/cos = T*rsq (DVE) in parallel;
    ssq and sincos are split per re/im pair so each starts as soon as
    its DFT copies land.
  * gate K-order [nyq3, mag_a, mag_b, sin_a, sin_b, cos_a, cos_b]:
    early-ready features first, so PE doesn't stall mid-accumulation.
  * deferred softmax normalization (as v1): inverse DFT runs on
    UNNORMALIZED U = T*e right after exp; the expsum->recip->broadcast
    chain overlaps it and only the final zo = psz*rb multiply needs rb.
  * all small M=1 matmul outputs (nyq re256 x2, gate row-256 x2,
    expsum x2) packed into disjoint partition rows of one shared PSUM
    tile: frees PSUM banks for cross-rep overlap.
  * act-table prefetch: dummy ops keyed to late-phase products trigger
    the sqrt<->exp table loads off the critical path.
"""

import numpy as np

B, H, F = 128, 512, 64
HID = 16
NF = H // 2 + 1          # 257
NCORES = 8
BL = B // NCORES         # 16 batch per core
BF = BL * F              # 1024 free columns per core
P = 128
CH = 512                 # free-dim chunk (8 batches x 64)
NCH = BF // CH           # 2


def _bf16():
    import ml_dtypes
    return ml_dtypes.bfloat16


def _build_constants(W_proj, b_proj, W_gate, b_gate):
    W_proj = np.asarray(W_proj, np.float64)
    b_proj = np.asarray(b_proj, np.float64)
    W_gate = np.asarray(W_gate, np.float64)
    b_gate = np.asarray(b_gate, np.float64)

    Wg = W_gate.reshape(NF, NF, HID)                      # [m, n, h]
    A = np.einsum("nch,mnh->ncm", W_proj, Wg)             # (257, 3, 257)
    bias_eff = b_gate + np.einsum("nh,mnh->m", b_proj, Wg)

    h = np.arange(H)
    n = np.arange(NF)
    ang = 2.0 * np.pi * np.outer(h, n) / H                # (512, 257)
    Cf = np.cos(ang)
    Sf = -np.sin(ang)

    # radix-2: even freqs contract with xp = x[:256]+x[256:], odd with
    # xm = x[:256]-x[256:]; freq tiles are parity-permuted everywhere:
    pa = np.arange(0, 256, 2)    # "a" tile: even freqs 0..254
    pb = np.arange(1, 256, 2)    # "b" tile: odd freqs 1..255
    # forward DFT weights (K = h<256), M-order [im_a im_b re_a re_b re256]
    Wf = np.concatenate(
        [Sf[0:256][:, pa], Sf[0:256][:, pb],
         Cf[0:256][:, pa], Cf[0:256][:, pb], Cf[0:256, 256:257]],
        axis=1,
    )                                                     # (256, 513)
    Wf_p = np.ascontiguousarray(
        Wf.reshape(2, P, 513).transpose(1, 0, 2)
    ).astype(_bf16())                                     # (P, 2, 513)

    # gate weights: K-tiles [mag_a mag_b sin_a sin_b cos_a cos_b |r| sgn].
    # Output m-columns are parity-permuted too, so the logit Mtiles line
    # up with the parity-ordered fft tiles in the U = T*e multiply.
    mperm = np.concatenate([pa, pb, [256]])
    Ap = np.zeros((8, P, NF))
    Ap[0] = A[pa, 0, :][:, mperm]
    Ap[1] = A[pb, 0, :][:, mperm]
    Ap[2] = A[pa, 1, :][:, mperm]
    Ap[3] = A[pb, 1, :][:, mperm]
    Ap[4] = A[pa, 2, :][:, mperm]
    Ap[5] = A[pb, 2, :][:, mperm]
    Ap[2, 0] = bias_eff[mperm]   # sin(freq 0) == 0: row rides the bias ones
    Ap[6, 0] = A[256, 0, mperm]  # rhs row = |re256|   (K=1)
    Ap[7, 0] = A[256, 2, mperm]  # rhs row = sign(re256) (K=1)
    Ap_p = np.ascontiguousarray(Ap.transpose(1, 0, 2)).astype(_bf16())

    # inverse DFT weights: K-tiles [re_a re_b (nyq+im_a) im_b]
    cn = np.full(NF, 2.0)
    cn[0] = 1.0
    cn[256] = 1.0
    Ci = np.cos(ang) * cn[None, :] / H                    # (512, 257)
    Si = (-2.0 / H) * np.sin(ang)
    Wi = np.zeros((4, P, H))
    Wi[0] = Ci[:, pa].T
    Wi[1] = Ci[:, pb].T
    Wi[2, 0] = Ci[:, 256]                                 # Nyquist row
    Wi[2, 1:128] = Si[:, pa[1:]].T
    Wi[3] = Si[:, pb].T
    Wi_p = np.ascontiguousarray(Wi.transpose(1, 0, 2)).astype(_bf16())
    return Wf_p, Ap_p, Wi_p


def _build_graph(reps=1):
    from contextlib import ExitStack

    import concourse.bass as bass  # noqa
    import concourse.tile as tile
    from concourse import bacc, mybir

    F32 = mybir.dt.float32
    BF16 = mybir.dt.bfloat16
    AF = mybir.ActivationFunctionType

    nc = bacc.Bacc(
        "TRN2",
        target_bir_lowering=False,
        debug=False,
        num_devices=NCORES,
    )

    # partition-major DRAM layouts -> fully contiguous DMAs
    x_ext = nc.dram_tensor("x", [P, 4, BL, F], BF16, kind="ExternalInput").ap()
    wf_ext = nc.dram_tensor("wf", [P, 2, 513], BF16, kind="ExternalInput").ap()
    ap_ext = nc.dram_tensor("apk", [P, 8, NF], BF16, kind="ExternalInput").ap()
    wi_ext = nc.dram_tensor("wi", [P, 4, H], BF16, kind="ExternalInput").ap()
    out_ext = nc.dram_tensor("out", [P, NCH, 4, CH], BF16,
                             kind="ExternalOutput").ap()

    with tile.TileContext(nc) as tc, ExitStack() as ctx:
        const = ctx.enter_context(tc.tile_pool(name="const", bufs=1))
        xpool = ctx.enter_context(tc.tile_pool(name="xp", bufs=2))
        tpool = ctx.enter_context(tc.tile_pool(name="tp", bufs=2))
        fpool = ctx.enter_context(tc.tile_pool(name="fe", bufs=2))
        wpool = ctx.enter_context(tc.tile_pool(name="work", bufs=2))
        opool = ctx.enter_context(tc.tile_pool(name="outs", bufs=2))
        psmm = ctx.enter_context(tc.tile_pool(name="psmm", bufs=2, space="PSUM"))
        psi = ctx.enter_context(tc.tile_pool(name="psi", bufs=2, space="PSUM"))
        psy = ctx.enter_context(tc.tile_pool(name="psy", bufs=2, space="PSUM"))
        pssm = ctx.enter_context(tc.tile_pool(name="pssm", bufs=2, space="PSUM"))

        # ---- constants (single contiguous DMAs, cold path only)
        wf_sb = const.tile([P, 2, 513], BF16, tag="wf", name="wf")
        nc.sync.dma_start(wf_sb[:], wf_ext)
        ap_sb = const.tile([P, 8, NF], BF16, tag="apk", name="apk")
        nc.scalar.dma_start(ap_sb[:], ap_ext)
        wi_sb = const.tile([P, 4, H], BF16, tag="wi", name="wi")
        nc.sync.dma_start(wi_sb[:], wi_ext)

        ones_bf = const.tile([P, 1], BF16, tag="ones_bf", name="ones_bf")
        nc.vector.memset(ones_bf[:], 1.0)
        ones_row = const.tile([1, P], BF16, tag="ones_row", name="ones_row")
        nc.vector.memset(ones_row[:], 1.0)
        ones_ch = const.tile([1, CH], BF16, tag="ones_ch", name="ones_ch")
        nc.vector.memset(ones_ch[:], 1.0)
        warm = const.tile([1, 8], F32, tag="warm", name="warm")
        nc.scalar.activation(warm[:], ones_ch[0:1, 0:8],
                             func=AF.Abs_reciprocal_sqrt)

        for _rep in range(reps):
            x_sb = xpool.tile([P, 4, BL, F], BF16, tag="x", name="x")
            nc.sync.dma_start(x_sb[:], x_ext)
            # radix-2 pre-combine on the (otherwise idle) Pool engine
            xp = xpool.tile([P, 2, BL, F], BF16, tag="xp", name="xp", bufs=2)
            xm = xpool.tile([P, 2, BL, F], BF16, tag="xm", name="xm", bufs=2)
            nc.gpsimd.tensor_add(xp[:], x_sb[:, 0:2], x_sb[:, 2:4])
            nc.gpsimd.tensor_sub(xm[:], x_sb[:, 0:2], x_sb[:, 2:4])

            fts, feats_l, r256s = [], [], []

            # ============ phase A: forward DFT + squares =========
            for c in range(NCH):
                bsl = slice(c * (CH // F), (c + 1) * (CH // F))
                # ft = [im_a im_b re_a re_b]
                ft = tpool.tile([P, 4, CH], BF16, tag=f"ft{c}", name=f"ft{c}")
                ssq = wpool.tile([P, 4, CH], BF16, tag=f"ssq{c}", name=f"ssq{c}")
                for mt in range(4):
                    src_t = [xp, xm, xp, xm][mt]    # even<-xp, odd<-xm
                    ps = psmm.tile([P, CH], F32, tag="mm", name="mm")
                    for k in range(2):
                        nc.tensor.matmul(
                            ps[:],
                            wf_sb[:, k, mt * P:(mt + 1) * P],
                            src_t[:, k, bsl, :],
                            start=(k == 0),
                            stop=(k == 1),
                        )
                    eng = [nc.vector, nc.scalar, nc.scalar, nc.scalar][mt]
                    if eng is nc.scalar:
                        nc.scalar.activation(ft[:, mt, :], ps[:], func=AF.Copy)
                    else:
                        nc.vector.tensor_copy(ft[:, mt, :], ps[:])
                    if mt % 2 == 1:
                        # square the (a,b) pair as soon as both copies land
                        nc.vector.tensor_mul(
                            ssq[:, mt - 1:mt + 1, :],
                            ft[:, mt - 1:mt + 1, :], ft[:, mt - 1:mt + 1, :])

                # nyquist re256 row (M=1, even freq)
                psn = pssm.tile([1, CH], F32, tag="sm", name="psn")
                for k in range(2):
                    nc.tensor.matmul(
                        psn[:],
                        wf_sb[:, k, 512:513],
                        xp[:, k, bsl, :],
                        start=(k == 0),
                        stop=(k == 1),
                    )
                t_abs = wpool.tile([1, CH], BF16, tag=f"tabs{c}", name=f"tabs{c}")
                nc.scalar.activation(t_abs[:], psn[:], func=AF.Abs)
                t_sgn = wpool.tile([1, CH], BF16, tag=f"tsgn{c}", name=f"tsgn{c}")
                nc.scalar.activation(t_sgn[:], psn[:], func=AF.Sign)
                r256 = wpool.tile([1, CH], BF16, tag=f"r256_{c}", name=f"r256_{c}")
                nc.scalar.activation(r256[:], psn[:], func=AF.Copy)
                r256s.append((r256, t_abs, t_sgn))

                # features: rsq = 1/sqrt(sq); mag = sq*rsq; sin/cos = T*rsq
                sq = wpool.tile([P, 2, CH], BF16, tag=f"sq{c}", name=f"sq{c}")
                nc.vector.tensor_add(sq[:], ssq[:, 0:2, :], ssq[:, 2:4, :])
                rsq = wpool.tile([P, 2, CH], BF16, tag=f"rsq{c}", name=f"rsq{c}")
                nc.scalar.activation(rsq[:], sq[:], func=AF.Abs_reciprocal_sqrt)
                feats = fpool.tile([P, 6, CH], BF16, tag=f"fe{c}", name=f"fe{c}")
                nc.vector.tensor_mul(feats[:, 0:2, :], sq[:], rsq[:])
                nc.vector.tensor_mul(feats[:, 2:4, :], ft[:, 0:2, :], rsq[:])
                nc.vector.tensor_mul(feats[:, 4:6, :], ft[:, 2:4, :], rsq[:])
                # bias rides the always-zero sin(freq0) feature row
                nc.vector.tensor_copy(feats[0:1, 2, :], ones_ch[:])
                fts.append(ft)
                feats_l.append(feats)


            # == phase B/C per chunk: logits, weights, u, inverse ==
            zo = opool.tile([P, NCH, 4, CH], BF16, tag="zo", name="zo", bufs=1)
            for c in range(NCH):
                ft, feats = fts[c], feats_l[c]
                r256, t_abs, t_sgn = r256s[c]
                et = wpool.tile([P, 2, CH], BF16, tag=f"et{c}", name=f"et{c}")
                e2 = wpool.tile([1, CH], BF16, tag=f"e2_{c}", name=f"e2_{c}")
                # gate K-order: early-ready nyquist K=1 rows first
                klist = [(6, t_abs[0:1, :], 1), (7, t_sgn[0:1, :], 1)] + [
                    (k, feats[:, k, :], P) for k in range(6)]
                for mt in range(3):
                    msl = slice(mt * P, NF if mt == 2 else (mt + 1) * P)
                    mp = 1 if mt == 2 else P
                    ps = (pssm.tile([1, CH], F32, tag="sm", name="psy2")[:]
                          if mt == 2 else
                          psy.tile([mp, CH], F32, tag="y", name="psy")[:])
                    for i, (k, rhs, kk) in enumerate(klist):
                        nc.tensor.matmul(
                            ps, ap_sb[0:kk, k, msl], rhs,
                            start=(i == 0), stop=(i == len(klist) - 1))
                    # e = exp(silu(y)) via tanh: exp(0.5*(1+tanh(y/2))*y)
                    th = wpool.tile([mp, CH], F32, tag=f"th{mt}", name=f"th{mt}",
                                    bufs=3)
                    nc.scalar.activation(th[:], ps, func=AF.Tanh, scale=0.5)
                    ysw = wpool.tile([mp, CH], F32, tag=f"ysw{mt}",
                                     name=f"ysw{mt}", bufs=3)
                    nc.vector.scalar_tensor_tensor(
                        out=ysw[:], in0=th[:], scalar=1.0, in1=ps,
                        op0=mybir.AluOpType.add, op1=mybir.AluOpType.mult,
                    )
                    tgt = e2[:] if mt == 2 else et[:, mt, :]
                    nc.scalar.activation(tgt, ysw[:], func=AF.Exp, scale=0.5)

                # unnormalized U = T*e right after exp (inverse DFT needn't
                # wait for the softmax sum)
                Ure = wpool.tile([P, 2, CH], BF16, tag=f"ur{c}", name=f"ur{c}")
                nc.vector.tensor_mul(Ure[:], ft[:, 2:4, :], et[:])
                Uim = wpool.tile([P, 2, CH], BF16, tag=f"ui{c}", name=f"ui{c}")
                nc.vector.tensor_mul(Uim[:], ft[:, 0:2, :], et[:])
                nc.vector.tensor_mul(Uim[0:1, 0, :], r256[:], e2[:])
                U = [Ure[:, 0, :], Ure[:, 1, :], Uim[:, 0, :], Uim[:, 1, :]]

                # exp-sum, reciprocal, broadcast (overlaps inverse DFT)
                ps_s = pssm.tile([1, CH], F32, tag="sm", name="ps_s")
                nc.tensor.matmul(ps_s[:], ones_bf[0:1, :], e2[:],
                                 start=True, stop=False)
                nc.tensor.matmul(ps_s[:], ones_bf[:], et[:, 0, :],
                                 start=False, stop=False)
                nc.tensor.matmul(ps_s[:], ones_bf[:], et[:, 1, :],
                                 start=False, stop=True)
                srec = wpool.tile([1, CH], BF16, tag=f"srec_{c}",
                                  name=f"srec_{c}")
                with nc.allow_low_precision(reason="bf16 softmax scale"):
                    nc.vector.reciprocal(srec[:], ps_s[:])
                ps_rb = psi.tile([P, CH], F32, tag="iv", name="ps_rb")
                nc.tensor.matmul(ps_rb[:], ones_row[:], srec[:], start=True,
                                 stop=True)
                rb = wpool.tile([P, CH], BF16, tag=f"rb_{c}", name=f"rb_{c}")
                nc.scalar.activation(rb[:], ps_rb[:], func=AF.Copy)

                # inverse DFT + deferred-normalization epilogue + out DMA
                ikorder = [0, 1, 3, 2]  # im_a (with late u256 row) last
                for mt in range(4):
                    ps = psi.tile([P, CH], F32, tag="iv", name="psz")
                    for j, k in enumerate(ikorder):
                        nc.tensor.matmul(
                            ps[:],
                            wi_sb[:, k, mt * P:(mt + 1) * P],
                            U[k],
                            start=(j == 0),
                            stop=(j == 3),
                        )
                    nc.vector.tensor_mul(zo[:, c, mt, :], ps[:], rb[:])
                nc.sync.dma_start(out_ext[:, c], zo[:, c])

            # table prefetch: next rep's rsq-set load keyed to last exp
            nc.scalar.activation(warm[:], e2[0:1, 0:8],
                                 func=AF.Abs_reciprocal_sqrt)

    nc.compile()
    return nc


_CACHE = {}


def _pack_in_maps(inputs):
    Wf, Ap, Wi = _build_constants(
        inputs["W_proj"], inputs["b_proj"], inputs["W_gate"], inputs["b_gate"]
    )
    x = np.ascontiguousarray(np.asarray(inputs["x"], np.float32))
    return [
        {
            # (BL,H,F) -> (P,4,BL,F): partition-major, fully contiguous
            "x": np.ascontiguousarray(
                x[c * BL:(c + 1) * BL].transpose(1, 0, 2)
                .reshape(4, P, BL, F).transpose(1, 0, 2, 3)
            ).astype(_bf16()),
            "wf": Wf,
            "apk": Ap,
            "wi": Wi,
        }
        for c in range(NCORES)
    ]


def _unpack_out(r):
    # (P,NCH,4,CH) -> (BL,H,F): h = mt*128+p, b = c*8+bb, col = bb*64+f
    o = np.asarray(r, dtype=np.float32)
    return o.reshape(P, NCH, 4, CH // F, F).transpose(1, 3, 2, 0, 4).reshape(
        BL, H, F)


def _run(inputs, trace=False):
    from concourse.bass_utils import run_bass_kernel_spmd

    if "graph" not in _CACHE:
        _CACHE["graph"] = _build_graph()
    nc = _CACHE["graph"]
    in_maps = _pack_in_maps(inputs)
    res = run_bass_kernel_spmd(nc, in_maps, core_ids=list(range(NCORES)),
                               trace=trace)
    out = np.concatenate([_unpack_out(r["out"]) for r in res.results], axis=0)
    return out.astype(np.float32), res


def kernel(**inputs):
    out, _ = _run(inputs, trace=False)
    return out


def _make_exec(nc):
    """Build a jit-cached 8-core executor for a compiled Bacc graph,
    replicating bass2jax.run_bass_via_pjrt's multi-core path but reusable
    across calls (for timing)."""
    import jax
    import numpy as np
    from jax.sharding import Mesh, PartitionSpec
    from jax.experimental.shard_map import shard_map
    from concourse import mybir
    from concourse.bass2jax import _bass_exec_p, install_neuronx_cc_hook

    install_neuronx_cc_hook()
    from concourse.bass2jax import partition_id_tensor

    n_cores = NCORES
    pid_name = nc.partition_id_tensor.name if nc.partition_id_tensor else None
    in_names, out_names, out_avals, zero_outs = [], [], [], []
    for alloc in nc.m.functions[0].allocations:
        if not isinstance(alloc, mybir.MemoryLocationSet):
            continue
        name = alloc.memorylocations[0].name
        if alloc.kind == "ExternalInput":
            if name != pid_name:
                in_names.append(name)
        elif alloc.kind == "ExternalOutput":
            out_names.append(name)
            shape = tuple(alloc.tensor_shape)
            dtype = mybir.dt.np(alloc.dtype)
            out_avals.append(jax.core.ShapedArray(shape, dtype))
            zero_outs.append(np.zeros(shape, dtype))
    n_params = len(in_names)
    all_names = in_names + out_names
    if pid_name is not None:
        all_names = all_names + [pid_name]

    def _body(*args):
        operands = list(args)
        if pid_name is not None:
            operands.append(partition_id_tensor())
        outs = _bass_exec_p.bind(
            *operands,
            out_avals=tuple(out_avals),
            in_names=tuple(all_names),
            out_names=tuple(out_names),
            lowering_input_output_aliases=(),
            sim_require_finite=True,
            sim_require_nnan=True,
            nc=nc,
        )
        return tuple(outs)

    devices = jax.devices()[:n_cores]
    mesh = Mesh(np.asarray(devices), ("core",))
    n_all = n_params + len(out_names)
    fn = jax.jit(
        shard_map(
            _body,
            mesh=mesh,
            in_specs=(PartitionSpec("core"),) * n_all,
            out_specs=(PartitionSpec("core"),) * len(out_names),
            check_rep=False,
        ),
        keep_unused=True,
    )

    def pack(in_maps):
        concat = [
            np.concatenate([np.asarray(in_maps[c][k]) for c in range(n_cores)], axis=0)
            for k in in_names
        ]
        concat += [
            np.zeros((n_cores * z.shape[0], *z.shape[1:]), z.dtype) for z in zero_outs
        ]
        return [jax.device_put(a) for a in concat]

    return fn, pack, out_names, out_avals


# revision 8
# speedup vs baseline: 3.8069x; 1.9028x over previous
"""AdaptiveSpectrumLayer Trainium2 kernel v3 (8-core data-parallel).

v2 -> v3 (critical-path fixes from TimelineSim gap analysis):
  * features via Abs_reciprocal_sqrt (direct 1/sqrt on Act): rsq feeds
    both mag = sq*rsq (DVE) and sin/cos = T*rsq (DVE) in parallel;
    ssq and sincos are split per re/im pair so each starts as soon as
    its DFT copies land.
  * gate K-order [nyq3, mag_a, mag_b, sin_a, sin_b, cos_a, cos_b]:
    early-ready features first, so PE doesn't stall mid-accumulation.
  * deferred softmax normalization (as v1): inverse DFT runs on
    UNNORMALIZED U = T*e right after exp; the expsum->recip->broadcast
    chain overlaps it and only the final zo = psz*rb multiply needs rb.
  * all small M=1 matmul outputs (nyq re256 x2, gate row-256 x2,
    expsum x2) packed into disjoint partition rows of one shared PSUM
    tile: frees PSUM banks for cross-rep overlap.
  * act-table prefetch: dummy ops keyed to late-phase products trigger
    the sqrt<->exp table loads off the critical path.
"""

import numpy as np

B, H, F = 128, 512, 64
HID = 16
NF = H // 2 + 1          # 257
NCORES = 8
BL = B // NCORES         # 16 batch per core
BF = BL * F              # 1024 free columns per core
P = 128
CH = 512                 # free-dim chunk (8 batches x 64)
NCH = BF // CH           # 2


def _bf16():
    import ml_dtypes
    return ml_dtypes.bfloat16


def _build_constants(W_proj, b_proj, W_gate, b_gate):
    W_proj = np.asarray(W_proj, np.float64)
    b_proj = np.asarray(b_proj, np.float64)
    W_gate = np.asarray(W_gate, np.float64)
    b_gate = np.asarray(b_gate, np.float64)

    Wg = W_gate.reshape(NF, NF, HID)                      # [m, n, h]
    A = np.einsum("nch,mnh->ncm", W_proj, Wg)             # (257, 3, 257)
    bias_eff = b_gate + np.einsum("nh,mnh->m", b_proj, Wg)

    h = np.arange(H)
    n = np.arange(NF)
    ang = 2.0 * np.pi * np.outer(h, n) / H                # (512, 257)
    Cf = np.cos(ang)
    Sf = -np.sin(ang)

    # radix-2: even freqs contract with xp = x[:256]+x[256:], odd with
    # xm = x[:256]-x[256:]; freq tiles are parity-permuted everywhere:
    pa = np.arange(0, 256, 2)    # "a" tile: even freqs 0..254
    pb = np.arange(1, 256, 2)    # "b" tile: odd freqs 1..255
    # forward DFT weights (K = h<256), M-order [im_a im_b re_a re_b re256]
    Wf = np.concatenate(
        [Sf[0:256][:, pa], Sf[0:256][:, pb],
         Cf[0:256][:, pa], Cf[0:256][:, pb], Cf[0:256, 256:257]],
        axis=1,
    )                                                     # (256, 513)
    Wf_p = np.ascontiguousarray(
        Wf.reshape(2, P, 513).transpose(1, 0, 2)
    ).astype(_bf16())                                     # (P, 2, 513)

    # gate weights: K-tiles [mag_a mag_b sin_a sin_b cos_a cos_b |r| sgn].
    # Output m-columns are parity-permuted too, so the logit Mtiles line
    # up with the parity-ordered fft tiles in the U = T*e multiply.
    mperm = np.concatenate([pa, pb, [256]])
    Ap = np.zeros((8, P, NF))
    Ap[0] = A[pa, 0, :][:, mperm]
    Ap[1] = A[pb, 0, :][:, mperm]
    Ap[2] = A[pa, 1, :][:, mperm]
    Ap[3] = A[pb, 1, :][:, mperm]
    Ap[4] = A[pa, 2, :][:, mperm]
    Ap[5] = A[pb, 2, :][:, mperm]
    Ap[2, 0] = bias_eff[mperm]   # sin(freq 0) == 0: row rides the bias ones
    Ap[6, 0] = A[256, 0, mperm]  # rhs row = |re256|   (K=1)
    Ap[7, 0] = A[256, 2, mperm]  # rhs row = sign(re256) (K=1)
    Ap_p = np.ascontiguousarray(Ap.transpose(1, 0, 2)).astype(_bf16())

    # inverse DFT weights: K-tiles [re_a re_b (nyq+im_a) im_b]
    cn = np.full(NF, 2.0)
    cn[0] = 1.0
    cn[256] = 1.0
    Ci = np.cos(ang) * cn[None, :] / H                    # (512, 257)
    Si = (-2.0 / H) * np.sin(ang)
    Wi = np.zeros((4, P, H))
    Wi[0] = Ci[:, pa].T
    Wi[1] = Ci[:, pb].T
    Wi[2, 0] = Ci[:, 256]                                 # Nyquist row
    Wi[2, 1:128] = Si[:, pa[1:]].T
    Wi[3] = Si[:, pb].T
    Wi_p = np.ascontiguousarray(Wi.transpose(1, 0, 2)).astype(_bf16())
    return Wf_p, Ap_p, Wi_p


def _build_graph(reps=1):
    from contextlib import ExitStack

    import concourse.bass as bass  # noqa
    import concourse.tile as tile
    from concourse import bacc, mybir

    F32 = mybir.dt.float32
    BF16 = mybir.dt.bfloat16
    AF = mybir.ActivationFunctionType

    nc = bacc.Bacc(
        "TRN2",
        target_bir_lowering=False,
        debug=False,
        num_devices=NCORES,
    )

    # partition-major DRAM layouts -> fully contiguous DMAs
    x_ext = nc.dram_tensor("x", [P, 4, BL, F], BF16, kind="ExternalInput").ap()
    wf_ext = nc.dram_tensor("wf", [P, 2, 513], BF16, kind="ExternalInput").ap()
    ap_ext = nc.dram_tensor("apk", [P, 8, NF], BF16, kind="ExternalInput").ap()
    wi_ext = nc.dram_tensor("wi", [P, 4, H], BF16, kind="ExternalInput").ap()
    out_ext = nc.dram_tensor("out", [P, NCH, 4, CH], BF16,
                             kind="ExternalOutput").ap()

    with tile.TileContext(nc) as tc, ExitStack() as ctx:
        const = ctx.enter_context(tc.tile_pool(name="const", bufs=1))
        xpool = ctx.enter_context(tc.tile_pool(name="xp", bufs=2))
        tpool = ctx.enter_context(tc.tile_pool(name="tp", bufs=2))
        fpool = ctx.enter_context(tc.tile_pool(name="fe", bufs=2))
        wpool = ctx.enter_context(tc.tile_pool(name="work", bufs=2))
        opool = ctx.enter_context(tc.tile_pool(name="outs", bufs=2))
        psmm = ctx.enter_context(tc.tile_pool(name="psmm", bufs=2, space="PSUM"))
        psi = ctx.enter_context(tc.tile_pool(name="psi", bufs=2, space="PSUM"))
        psy = ctx.enter_context(tc.tile_pool(name="psy", bufs=2, space="PSUM"))
        pssm = ctx.enter_context(tc.tile_pool(name="pssm", bufs=2, space="PSUM"))

        # ---- constants (single contiguous DMAs, cold path only)
        wf_sb = const.tile([P, 2, 513], BF16, tag="wf", name="wf")
        nc.sync.dma_start(wf_sb[:], wf_ext)
        ap_sb = const.tile([P, 8, NF], BF16, tag="apk", name="apk")
        nc.scalar.dma_start(ap_sb[:], ap_ext)
        wi_sb = const.tile([P, 4, H], BF16, tag="wi", name="wi")
        nc.sync.dma_start(wi_sb[:], wi_ext)

        ones_bf = const.tile([P, 1], BF16, tag="ones_bf", name="ones_bf")
        nc.vector.memset(ones_bf[:], 1.0)
        ones_row = const.tile([1, P], BF16, tag="ones_row", name="ones_row")
        nc.vector.memset(ones_row[:], 1.0)
        ones_ch = const.tile([1, CH], BF16, tag="ones_ch", name="ones_ch")
        nc.vector.memset(ones_ch[:], 1.0)
        warm = const.tile([1, 8], F32, tag="warm", name="warm")
        nc.scalar.activation(warm[:], ones_ch[0:1, 0:8],
                             func=AF.Abs_reciprocal_sqrt)

        for _rep in range(reps):
            x_sb = xpool.tile([P, 4, BL, F], BF16, tag="x", name="x")
            nc.sync.dma_start(x_sb[:], x_ext)
            # radix-2 pre-combine on the (otherwise idle) Pool engine
            xp = xpool.tile([P, 2, BL, F], BF16, tag="xp", name="xp", bufs=2)
            xm = xpool.tile([P, 2, BL, F], BF16, tag="xm", name="xm", bufs=2)
            nc.gpsimd.tensor_add(xp[:], x_sb[:, 0:2], x_sb[:, 2:4])
            nc.gpsimd.tensor_sub(xm[:], x_sb[:, 0:2], x_sb[:, 2:4])

            fts, feats_l, r256s = [], [], []

            # ============ phase A: forward DFT + squares =========
            for c in range(NCH):
                bsl = slice(c * (CH // F), (c + 1) * (CH // F))
                # ft = [im_a im_b re_a re_b]
                ft = tpool.tile([P, 4, CH], BF16, tag=f"ft{c}", name=f"ft{c}")
                ssq = wpool.tile([P, 4, CH], BF16, tag=f"ssq{c}", name=f"ssq{c}")
                for mt in range(4):
                    src_t = [xp, xm, xp, xm][mt]    # even<-xp, odd<-xm
                    ps = psmm.tile([P, CH], F32, tag="mm", name="mm")
                    for k in range(2):
                        nc.tensor.matmul(
                            ps[:],
                            wf_sb[:, k, mt * P:(mt + 1) * P],
                            src_t[:, k, bsl, :],
                            start=(k == 0),
                            stop=(k == 1),
                        )
                    eng = [nc.vector, nc.scalar, nc.scalar, nc.scalar][mt]
                    if eng is nc.scalar:
                        nc.scalar.activation(ft[:, mt, :], ps[:], func=AF.Copy)
                    else:
                        nc.vector.tensor_copy(ft[:, mt, :], ps[:])
                    if mt % 2 == 1:
                        # square the (a,b) pair as soon as both copies land
                        nc.vector.tensor_mul(
                            ssq[:, mt - 1:mt + 1, :],
                            ft[:, mt - 1:mt + 1, :], ft[:, mt - 1:mt + 1, :])

                # nyquist re256 row (M=1, even freq)
                psn = pssm.tile([1, CH], F32, tag="sm", name="psn")
                for k in range(2):
                    nc.tensor.matmul(
                        psn[:],
                        wf_sb[:, k, 512:513],
                        xp[:, k, bsl, :],
                        start=(k == 0),
                        stop=(k == 1),
                    )
                t_abs = wpool.tile([1, CH], BF16, tag=f"tabs{c}", name=f"tabs{c}")
                nc.scalar.activation(t_abs[:], psn[:], func=AF.Abs)
                t_sgn = wpool.tile([1, CH], BF16, tag=f"tsgn{c}", name=f"tsgn{c}")
                nc.scalar.activation(t_sgn[:], psn[:], func=AF.Sign)
                r256 = wpool.tile([1, CH], BF16, tag=f"r256_{c}", name=f"r256_{c}")
                nc.scalar.activation(r256[:], psn[:], func=AF.Copy)
                r256s.append((r256, t_abs, t_sgn))

                # features: rsq = 1/sqrt(sq); mag = sq*rsq; sin/cos = T*rsq
                sq = wpool.tile([P, 2, CH], BF16, tag=f"sq{c}", name=f"sq{c}")
                nc.vector.tensor_add(sq[:], ssq[:, 0:2, :], ssq[:, 2:4, :])
                rsq = wpool.tile([P, 2, CH], BF16, tag=f"rsq{c}", name=f"rsq{c}")
                nc.scalar.activation(rsq[:], sq[:], func=AF.Abs_reciprocal_sqrt)
                feats = fpool.tile([P, 6, CH], BF16, tag=f"fe{c}", name=f"fe{c}")
                nc.vector.tensor_mul(feats[:, 0:2, :], sq[:], rsq[:])
                nc.vector.tensor_mul(feats[:, 2:4, :], ft[:, 0:2, :], rsq[:])
                nc.vector.tensor_mul(feats[:, 4:6, :], ft[:, 2:4, :], rsq[:])
                # bias rides the always-zero sin(freq0) feature row
                nc.vector.tensor_copy(feats[0:1, 2, :], ones_ch[:])
                fts.append(ft)
                feats_l.append(feats)


            # act-table fence: a zero bias tile derived from the last rsq
            # output forces every exp-set op after both sqrt-set ops
            zb = wpool.tile([P, 1], F32, tag="zb", name="zb")
            nc.vector.tensor_scalar(
                out=zb[:], in0=feats_l[1][:, 0, 0:1], scalar1=0.0,
                scalar2=None, op0=mybir.AluOpType.mult)

            # == phase B/C per chunk: logits, weights, u, inverse ==
            zo = opool.tile([P, NCH, 4, CH], BF16, tag="zo", name="zo", bufs=1)
            for c in range(NCH):
                ft, feats = fts[c], feats_l[c]
                r256, t_abs, t_sgn = r256s[c]
                et = wpool.tile([P, 2, CH], BF16, tag=f"et{c}", name=f"et{c}")
                e2 = wpool.tile([1, CH], BF16, tag=f"e2_{c}", name=f"e2_{c}")
                # gate K-order: early-ready nyquist K=1 rows first
                klist = [(6, t_abs[0:1, :], 1), (7, t_sgn[0:1, :], 1)] + [
                    (k, feats[:, k, :], P) for k in range(6)]
                for mt in range(3):
                    msl = slice(mt * P, NF if mt == 2 else (mt + 1) * P)
                    mp = 1 if mt == 2 else P
                    ps = (pssm.tile([1, CH], F32, tag="sm", name="psy2")[:]
                          if mt == 2 else
                          psy.tile([mp, CH], F32, tag="y", name="psy")[:])
                    for i, (k, rhs, kk) in enumerate(klist):
                        nc.tensor.matmul(
                            ps, ap_sb[0:kk, k, msl], rhs,
                            start=(i == 0), stop=(i == len(klist) - 1))
                    # e = exp(silu(y)) via tanh: exp(0.5*(1+tanh(y/2))*y)
                    th = wpool.tile([mp, CH], F32, tag=f"th{mt}", name=f"th{mt}",
                                    bufs=3)
                    nc.scalar.activation(th[:], ps, func=AF.Tanh, scale=0.5,
                                         bias=zb[0:mp, 0:1])
                    ysw = wpool.tile([mp, CH], F32, tag=f"ysw{mt}",
                                     name=f"ysw{mt}", bufs=3)
                    nc.vector.scalar_tensor_tensor(
                        out=ysw[:], in0=th[:], scalar=1.0, in1=ps,
                        op0=mybir.AluOpType.add, op1=mybir.AluOpType.mult,
                    )
                    tgt = e2[:] if mt == 2 else et[:, mt, :]
                    nc.scalar.activation(tgt, ysw[:], func=AF.Exp, scale=0.5)

                # unnormalized U = T*e right after exp (inverse DFT needn't
                # wait for the softmax sum)
                Ure = wpool.tile([P, 2, CH], BF16, tag=f"ur{c}", name=f"ur{c}")
                nc.vector.tensor_mul(Ure[:], ft[:, 2:4, :], et[:])
                Uim = wpool.tile([P, 2, CH], BF16, tag=f"ui{c}", name=f"ui{c}")
                nc.vector.tensor_mul(Uim[:], ft[:, 0:2, :], et[:])
                nc.vector.tensor_mul(Uim[0:1, 0, :], r256[:], e2[:])
                U = [Ure[:, 0, :], Ure[:, 1, :], Uim[:, 0, :], Uim[:, 1, :]]

                # exp-sum, reciprocal, broadcast (overlaps inverse DFT)
                ps_s = pssm.tile([1, CH], F32, tag="sm", name="ps_s")
                nc.tensor.matmul(ps_s[:], ones_bf[0:1, :], e2[:],
                                 start=True, stop=False)
                nc.tensor.matmul(ps_s[:], ones_bf[:], et[:, 0, :],
                                 start=False, stop=False)
                nc.tensor.matmul(ps_s[:], ones_bf[:], et[:, 1, :],
                                 start=False, stop=True)
                srec = wpool.tile([1, CH], BF16, tag=f"srec_{c}",
                                  name=f"srec_{c}")
                with nc.allow_low_precision(reason="bf16 softmax scale"):
                    nc.vector.reciprocal(srec[:], ps_s[:])
                ps_rb = psi.tile([P, CH], F32, tag="iv", name="ps_rb")
                nc.tensor.matmul(ps_rb[:], ones_row[:], srec[:], start=True,
                                 stop=True)
                rb = wpool.tile([P, CH], BF16, tag=f"rb_{c}", name=f"rb_{c}")
                nc.scalar.activation(rb[:], ps_rb[:], func=AF.Copy)

                # inverse DFT + deferred-normalization epilogue + out DMA
                ikorder = [0, 1, 3, 2]  # im_a (with late u256 row) last
                for mt in range(4):
                    ps = psi.tile([P, CH], F32, tag="iv", name="psz")
                    for j, k in enumerate(ikorder):
                        nc.tensor.matmul(
                            ps[:],
                            wi_sb[:, k, mt * P:(mt + 1) * P],
                            U[k],
                            start=(j == 0),
                            stop=(j == 3),
                        )
                    nc.vector.tensor_mul(zo[:, c, mt, :], ps[:], rb[:])
                nc.sync.dma_start(out_ext[:, c], zo[:, c])

            # table prefetch: next rep's rsq-set load keyed to last exp
            nc.scalar.activation(warm[:], e2[0:1, 0:8],
                                 func=AF.Abs_reciprocal_sqrt)

    nc.compile()
    return nc


_CACHE = {}


def _pack_in_maps(inputs):
    Wf, Ap, Wi = _build_constants(
        inputs["W_proj"], inputs["b_proj"], inputs["W_gate"], inputs["b_gate"]
    )
    x = np.ascontiguousarray(np.asarray(inputs["x"], np.float32))
    return [
        {
            # (BL,H,F) -> (P,4,BL,F): partition-major, fully contiguous
            "x": np.ascontiguousarray(
                x[c * BL:(c + 1) * BL].transpose(1, 0, 2)
                .reshape(4, P, BL, F).transpose(1, 0, 2, 3)
            ).astype(_bf16()),
            "wf": Wf,
            "apk": Ap,
            "wi": Wi,
        }
        for c in range(NCORES)
    ]


def _unpack_out(r):
    # (P,NCH,4,CH) -> (BL,H,F): h = mt*128+p, b = c*8+bb, col = bb*64+f
    o = np.asarray(r, dtype=np.float32)
    return o.reshape(P, NCH, 4, CH // F, F).transpose(1, 3, 2, 0, 4).reshape(
        BL, H, F)


def _run(inputs, trace=False):
    from concourse.bass_utils import run_bass_kernel_spmd

    if "graph" not in _CACHE:
        _CACHE["graph"] = _build_graph()
    nc = _CACHE["graph"]
    in_maps = _pack_in_maps(inputs)
    res = run_bass_kernel_spmd(nc, in_maps, core_ids=list(range(NCORES)),
                               trace=trace)
    out = np.concatenate([_unpack_out(r["out"]) for r in res.results], axis=0)
    return out.astype(np.float32), res


def kernel(**inputs):
    out, _ = _run(inputs, trace=False)
    return out


def _make_exec(nc):
    """Build a jit-cached 8-core executor for a compiled Bacc graph,
    replicating bass2jax.run_bass_via_pjrt's multi-core path but reusable
    across calls (for timing)."""
    import jax
    import numpy as np
    from jax.sharding import Mesh, PartitionSpec
    from jax.experimental.shard_map import shard_map
    from concourse import mybir
    from concourse.bass2jax import _bass_exec_p, install_neuronx_cc_hook

    install_neuronx_cc_hook()
    from concourse.bass2jax import partition_id_tensor

    n_cores = NCORES
    pid_name = nc.partition_id_tensor.name if nc.partition_id_tensor else None
    in_names, out_names, out_avals, zero_outs = [], [], [], []
    for alloc in nc.m.functions[0].allocations:
        if not isinstance(alloc, mybir.MemoryLocationSet):
            continue
        name = alloc.memorylocations[0].name
        if alloc.kind == "ExternalInput":
            if name != pid_name:
                in_names.append(name)
        elif alloc.kind == "ExternalOutput":
            out_names.append(name)
            shape = tuple(alloc.tensor_shape)
            dtype = mybir.dt.np(alloc.dtype)
            out_avals.append(jax.core.ShapedArray(shape, dtype))
            zero_outs.append(np.zeros(shape, dtype))
    n_params = len(in_names)
    all_names = in_names + out_names
    if pid_name is not None:
        all_names = all_names + [pid_name]

    def _body(*args):
        operands = list(args)
        if pid_name is not None:
            operands.append(partition_id_tensor())
        outs = _bass_exec_p.bind(
            *operands,
            out_avals=tuple(out_avals),
            in_names=tuple(all_names),
            out_names=tuple(out_names),
            lowering_input_output_aliases=(),
            sim_require_finite=True,
            sim_require_nnan=True,
            nc=nc,
        )
        return tuple(outs)

    devices = jax.devices()[:n_cores]
    mesh = Mesh(np.asarray(devices), ("core",))
    n_all = n_params + len(out_names)
    fn = jax.jit(
        shard_map(
            _body,
            mesh=mesh,
            in_specs=(PartitionSpec("core"),) * n_all,
            out_specs=(PartitionSpec("core"),) * len(out_names),
            check_rep=False,
        ),
        keep_unused=True,
    )

    def pack(in_maps):
        concat = [
            np.concatenate([np.asarray(in_maps[c][k]) for c in range(n_cores)], axis=0)
            for k in in_names
        ]
        concat += [
            np.zeros((n_cores * z.shape[0], *z.shape[1:]), z.dtype) for z in zero_outs
        ]
        return [jax.device_put(a) for a in concat]

    return fn, pack, out_names, out_avals
